# revision 7
# baseline (speedup 1.0000x reference)
"""Trainium2 Bass kernel for nn_LocallyConnectedAutoencoder.

Reference computation (per sample, image H=256 x W=128, 32x32 patches):
  patch t=(ph,pw):  enc[t] = x_patch[t] @ We[t].T + eb[t]      (1024 -> 32)
                    dec[t] = enc[t] @ Wd[t].T + db[t]          (32 -> 1024)
  out = sigmoid(dec), patches scattered back to image layout.

Strategy (pure data parallel, batch 2048 sharded 8 ways -> 256/core):
  - Host casts x to bf16 (halves input DMA traffic; matmuls accumulate fp32).
  - DMA-transpose (xbar) loads x as (c_full=128 partitions, (b, row) free),
    so the patch-dim contraction lands on partitions with zero PE transposes.
  - Encode: for each patch-row ph and image row r, the 4 patches (pw=0..3)
    are 32x32 matmuls placed at PE tile_position (32pw, 32pw) -> they run
    concurrently in the 128x128 array; PSUM accumulates over the 32 rows.
  - Decode: per patch, (33->128b x 512) matmuls from the encoded SBUF tile.
  - ScalarE applies sigmoid straight out of PSUM, scattering (r, c) blocks
    into a (128b, 4096) row-block tile; one contiguous 2MB DMA per
    (batch-tile, ph) stores the result.
"""

import sys

sys.path.insert(0, "/opt/trn_rl_repo")

from contextlib import ExitStack

import ml_dtypes
import numpy as np

import concourse.bass as bass
import concourse.tile as tile
from concourse import bacc, mybir
from concourse.bass_utils import run_bass_kernel_spmd

H, W, P = 256, 128, 32
NPH, NPW = H // P, W // P          # 8, 4
TP, PD, HPP = NPH * NPW, P * P, 32  # 32 patches, 1024 patch dim, 32 hidden
N_CORES = 8
BPC = 2048 // N_CORES              # 256 samples per core
BT = 128                           # batch tile (partition dim)
NBT = BPC // BT                    # 2 batch tiles per core

BF16 = ml_dtypes.bfloat16
DT = mybir.dt

_BUILD_CACHE: dict = {}


def _build_bass(has_db: bool) -> bass.Bass:
    nc = bacc.Bacc("TRN2", target_bir_lowering=False, debug=False)

    x_d = nc.dram_tensor("x", [BPC, H * W], DT.bfloat16, kind="ExternalInput").ap()
    wep_d = nc.dram_tensor("wep", [128, NPH * PD], DT.bfloat16, kind="ExternalInput").ap()
    wdp_d = nc.dram_tensor("wdp", [128, NPH * PD], DT.bfloat16, kind="ExternalInput").ap()
    ebp_d = nc.dram_tensor("ebp", [128, NPH], DT.float32, kind="ExternalInput").ap()
    if has_db:
        db_d = nc.dram_tensor("db", [1, TP * PD], DT.bfloat16, kind="ExternalInput").ap()
    out_d = nc.dram_tensor("out", [BPC, H * W], DT.float32, kind="ExternalOutput").ap()

    sigmoid = mybir.ActivationFunctionType.Sigmoid
    identity = mybir.ActivationFunctionType.Identity

    with tile.TileContext(nc) as tc, ExitStack() as ctx:
        wpool = ctx.enter_context(tc.tile_pool(name="weights", bufs=1))
        xpool = ctx.enter_context(tc.tile_pool(name="xT", bufs=1))
        enc_ps_pool = ctx.enter_context(tc.tile_pool(name="encps", bufs=2, space="PSUM"))
        dec_ps_pool = ctx.enter_context(tc.tile_pool(name="decps", bufs=4, space="PSUM"))
        enc_sb_pool = ctx.enter_context(tc.tile_pool(name="encsb", bufs=3))
        out_pool = ctx.enter_context(tc.tile_pool(name="out", bufs=2))

        wep = wpool.tile([128, NPH * PD], DT.bfloat16)
        nc.sync.dma_start(wep[:], wep_d[:])
        wdp = wpool.tile([128, NPH * PD], DT.bfloat16)
        nc.sync.dma_start(wdp[:], wdp_d[:])
        ebp = wpool.tile([128, NPH], DT.float32)
        nc.sync.dma_start(ebp[:], ebp_d[:])
        if has_db:
            dbt = wpool.tile([1, TP * PD], DT.bfloat16)
            nc.sync.dma_start(dbt[:], db_d[:])
            ones = wpool.tile([1, 128], DT.bfloat16)
            nc.vector.memset(ones[:], 1.0)

        # Transposed x, one tile per batch tile of 128 samples.
        # Free layout: (b, j) with j = ph*32 + r (image row), partition = c_full.
        xts = []
        for bt in range(NBT):
            xt = xpool.tile([128, BT * H], DT.bfloat16, tag=f"xt{bt}")
            src = x_d[bt * BT:(bt + 1) * BT, :].rearrange("b (j c) -> (b j) c", c=128)
            nc.sync.dma_start(xt[:], src, transpose=True)
            xts.append(xt)

        def encode(bt: int, ph: int):
            vx = xts[bt][:].rearrange("p (b j) -> p b j", j=H)
            enc_ps = enc_ps_pool.tile([128, BT], DT.float32)
            for r in range(P):
                for pw in range(NPW):
                    nc.tensor.matmul(
                        enc_ps[32 * pw:32 * (pw + 1), :],
                        lhsT=wep[32 * pw:32 * (pw + 1),
                                 ph * PD + r * 32:ph * PD + r * 32 + 32],
                        rhs=vx[32 * pw:32 * (pw + 1), :, ph * 32 + r],
                        start=(r == 0),
                        stop=(r == P - 1),
                        tile_position=(32 * pw, 32 * pw),
                        # The 4 pw-groups occupy disjoint 32-partition slices
                        # of one PSUM bank; the group tracker models the bank
                        # as a single zero region, so silence it.
                        skip_group_check=True,
                    )
            enc_sb = enc_sb_pool.tile([128, BT], DT.bfloat16)
            nc.scalar.activation(enc_sb[:], enc_ps[:], identity,
                                 bias=ebp[:, ph:ph + 1])
            return enc_sb

        def decode(bt: int, ph: int, enc_sb):
            out_t = out_pool.tile([128, NPW * PD], DT.float32)
            ov = out_t[:].rearrange("p (r pw c) -> p r pw c", pw=NPW, c=32)
            for pw in range(NPW):
                t = ph * NPW + pw
                for half in range(2):
                    dec_ps = dec_ps_pool.tile([128, 512], DT.float32)
                    if has_db:
                        nc.tensor.matmul(
                            dec_ps[:],
                            lhsT=ones[:, :],
                            rhs=dbt[0:1, t * PD + half * 512:t * PD + (half + 1) * 512],
                            start=True, stop=False,
                        )
                    nc.tensor.matmul(
                        dec_ps[:],
                        lhsT=enc_sb[32 * pw:32 * (pw + 1), :],
                        rhs=wdp[32 * pw:32 * (pw + 1),
                                ph * PD + half * 512:ph * PD + (half + 1) * 512],
                        start=not has_db, stop=True,
                        tile_position=(32 * pw, 0),
                    )
                    nc.scalar.activation(
                        ov[:, half * 16:(half + 1) * 16, pw, :],
                        dec_ps[:].rearrange("p (r c) -> p r c", c=32),
                        sigmoid,
                    )
            nc.sync.dma_start(
                out_d[bt * BT:(bt + 1) * BT, ph * NPW * PD:(ph + 1) * NPW * PD],
                out_t[:],
            )

        # Software-pipelined: decode of iteration i-1 is traced after encode of
        # iteration i so the PE never stalls on the ScalarE PSUM->SBUF copy.
        pending = None
        for bt in range(NBT):
            for ph in range(NPH):
                enc_sb = encode(bt, ph)
                if pending is not None:
                    decode(*pending)
                pending = (bt, ph, enc_sb)
        decode(*pending)

    nc.compile()
    return nc


def _pack_params(encoder_weights, encoder_bias, decoder_weights, decoder_bias):
    we = np.asarray(encoder_weights, np.float32)   # (32t, 32h, 1024p)
    wd = np.asarray(decoder_weights, np.float32)   # (32t, 1024p, 32h)
    eb = np.asarray(encoder_bias, np.float32)      # (32t, 32h)
    db = np.asarray(decoder_bias, np.float32)      # (32t, 1024p)

    # wep[(pw,c), (ph,r,h)] = we[ph*4+pw, h, r*32+c]
    w5 = we.reshape(NPH, NPW, HPP, P, P)                      # ph pw h r c
    wep = np.ascontiguousarray(w5.transpose(1, 4, 0, 3, 2)).reshape(128, NPH * PD)
    # wdp[(pw,h), (ph,p')] = wd[ph*4+pw, p', h]
    d4 = wd.reshape(NPH, NPW, PD, HPP)                        # ph pw p' h
    wdp = np.ascontiguousarray(d4.transpose(1, 3, 0, 2)).reshape(128, NPH * PD)
    # ebp[(pw,h), ph] = eb[ph*4+pw, h]
    e3 = eb.reshape(NPH, NPW, HPP)                            # ph pw h
    ebp = np.ascontiguousarray(e3.transpose(1, 2, 0)).reshape(128, NPH)

    has_db = bool(np.any(db))
    return (wep.astype(BF16), wdp.astype(BF16), np.ascontiguousarray(ebp),
            db.reshape(1, TP * PD).astype(BF16), has_db)


def kernel(x, encoder_weights, encoder_bias, decoder_weights, decoder_bias):
    x = np.asarray(x)
    orig_shape = x.shape
    xf = np.ascontiguousarray(x, dtype=np.float32).reshape(2048, H * W)
    xb = xf.astype(BF16)

    wep, wdp, ebp, db, has_db = _pack_params(
        encoder_weights, encoder_bias, decoder_weights, decoder_bias)

    if has_db not in _BUILD_CACHE:
        _BUILD_CACHE[has_db] = _build_bass(has_db)
    nc = _BUILD_CACHE[has_db]

    in_maps = []
    for i in range(N_CORES):
        m = {
            "x": xb[i * BPC:(i + 1) * BPC],
            "wep": wep,
            "wdp": wdp,
            "ebp": ebp,
        }
        if has_db:
            m["db"] = db
        in_maps.append(m)

    res = run_bass_kernel_spmd(nc, in_maps, list(range(N_CORES)))
    out = np.concatenate([res.results[i]["out"] for i in range(N_CORES)], axis=0)
    return out.reshape(orig_shape).astype(np.float32)
